# revision 4
# baseline (speedup 1.0000x reference)
"""v8: bf16 conv; head-critical DMAs front-loaded on the sync queue.

The first matmul needs xa rows 0..17 and w[k=0..2]; those go first on
the sync HWDGE queue (which starts draining ~1.9us before scalar's):
xa0A(18 rows), w0, w1, w2. Remaining loads (b, xa0B rows 16..25 -- two
rows duplicated so each tile's rhs stays contiguous -- and xb) ride the
scalar queue. k=3..8 matmuls consume w1/w2 as they land. Warmup 12->8
since real work now starts ~10.2us. Final (img3,t6,h1) tile drains as
224+112+112 col pieces to shorten the post-matmul tail.
"""

import sys

if "/opt/trn_rl_repo" not in sys.path:
    sys.path.insert(0, "/opt/trn_rl_repo")

import ml_dtypes
import numpy as np

N, C_IN, H, W = 32, 128, 56, 56
C_OUT, KH, KW = 256, 3, 3
N_CORES = 8
IMGS = N // N_CORES
HP, WP = H + 2, W + 2
RPT = 8
NT = H // RPT          # 7
TF = RPT * W           # 448
NH = C_OUT // 128      # 2

# xaA rows [0,18) serves tiles t=0,1; xaB rows [16,26) serves t=2
# (rows 16,17 duplicated); xb rows [24,58) serves t=3..6
XAA_R0, XAA_R1 = 0, 18
XAB_R0, XAB_R1 = 16, 26
XB_R0, XB_R1 = 24, 58
N_WARMUP_MM = 8

_CACHE = {}


def _build_program():
    import concourse.mybir as mybir
    import concourse.tile as tile
    from concourse import bacc

    F32 = mybir.dt.float32
    BF16 = mybir.dt.bfloat16

    nc = bacc.Bacc("TRN2", target_bir_lowering=False, debug=False,
                   enable_asserts=False)

    xp = nc.dram_tensor("xp", [IMGS, C_IN, HP, WP], BF16,
                        kind="ExternalInput").ap()
    w = nc.dram_tensor("w", [C_IN, KH * KW, C_OUT], BF16,
                       kind="ExternalInput").ap()
    b = nc.dram_tensor("b", [128, NH], F32, kind="ExternalInput").ap()
    out = nc.dram_tensor("out", [IMGS, C_OUT, H, W], F32,
                         kind="ExternalOutput").ap()
    out_v = out.rearrange("n c a b -> n c (a b)")

    with tile.TileContext(nc) as tc:
        with (
            tc.tile_pool(name="consts", bufs=1) as consts,
            tc.tile_pool(name="xin", bufs=1) as xin,
            tc.tile_pool(name="outp", bufs=2) as outp,
            tc.tile_pool(name="psum", bufs=7, space="PSUM") as psum,
        ):
            scratch = consts.tile([128, TF], BF16, tag="scratch")
            nc.vector.memset(scratch[:], 0.0)

            # sync queue, in packet-drain order: xa0A then the 3 w slices
            xa0A = xin.tile([C_IN, XAA_R1 - XAA_R0, WP], BF16, tag="xaA")
            nc.sync.dma_start(out=xa0A[:], in_=xp[0, :, XAA_R0:XAA_R1])
            w_sb = []
            for j in range(3):
                wj = consts.tile([C_IN, 3, C_OUT], BF16, tag=f"w{j}")
                nc.sync.dma_start(out=wj[:], in_=w[:, 3 * j:3 * j + 3])
                w_sb.append(wj)

            # scalar queue: everything with slack
            b_sb = consts.tile([128, NH], F32, tag="b")
            nc.scalar.dma_start(out=b_sb[:], in_=b)
            xa0B = xin.tile([C_IN, XAB_R1 - XAB_R0, WP], BF16, tag="xaB")
            nc.scalar.dma_start(out=xa0B[:], in_=xp[0, :, XAB_R0:XAB_R1])
            xb0 = xin.tile([C_IN, XB_R1 - XB_R0, WP], BF16, tag="xb")
            nc.scalar.dma_start(out=xb0[:], in_=xp[0, :, XB_R0:XB_R1])
            xts = {0: (xa0A, xa0B, xb0)}
            for img in range(1, IMGS):
                xaA = xin.tile([C_IN, XAA_R1 - XAA_R0, WP], BF16, tag="xaA")
                nc.scalar.dma_start(out=xaA[:], in_=xp[img, :, XAA_R0:XAA_R1])
                xaB = xin.tile([C_IN, XAB_R1 - XAB_R0, WP], BF16, tag="xaB")
                nc.scalar.dma_start(out=xaB[:], in_=xp[img, :, XAB_R0:XAB_R1])
                xb = xin.tile([C_IN, XB_R1 - XB_R0, WP], BF16, tag="xb")
                nc.scalar.dma_start(out=xb[:], in_=xp[img, :, XB_R0:XB_R1])
                xts[img] = (xaA, xaB, xb)

            warm_ps = psum.tile([128, TF], F32, tag="warm", bufs=1)
            for _ in range(N_WARMUP_MM):
                nc.tensor.matmul(warm_ps[:, :], lhsT=scratch[:, :128],
                                 rhs=scratch[:, :], start=True, stop=True)

            def conv_tile(src, r0, kcol0, rows, pt_cols):
                """9-matmul accumulation for `rows` output H-rows."""
                pt = psum.tile([128, pt_cols], F32, tag="pt")
                for k in range(KH * KW):
                    kh, kw = divmod(k, KW)
                    nc.tensor.matmul(
                        pt[:, :rows * W],
                        lhsT=w_sb[k // 3][:, k % 3, kcol0:kcol0 + 128],
                        rhs=src[:, r0 + kh:r0 + kh + rows, kw:kw + W],
                        start=(k == 0),
                        stop=(k == KH * KW - 1),
                    )
                return pt

            for img in range(IMGS):
                xaA, xaB, xb = xts[img]
                ots = [outp.tile([128, H * W], F32, tag=f"ot{h}",
                                 name=f"ot{img}_{h}")
                       for h in range(NH)]
                for t in range(NT):
                    if t < 2:
                        src, r_off = xaA, XAA_R0
                    elif t == 2:
                        src, r_off = xaB, XAB_R0
                    else:
                        src, r_off = xb, XB_R0
                    r0 = RPT * t - r_off
                    for h in range(NH):
                        last = (img == IMGS - 1 and t == NT - 1 and h == NH - 1)
                        # final tile in 4+2+2 row pieces: shorter tail drain
                        row_parts = [4, 2, 2] if last else [RPT]
                        rr = 0
                        for rows in row_parts:
                            pt = conv_tile(src, r0 + rr, h * 128,
                                           rows, rows * W)
                            c0 = t * TF + rr * W
                            nc.vector.tensor_scalar_add(
                                out=ots[h][:, c0:c0 + rows * W],
                                in0=pt[:, :rows * W],
                                scalar1=b_sb[:, h:h + 1],
                            )
                            nc.sync.dma_start(
                                out=out_v[img, h * 128:(h + 1) * 128,
                                          c0:c0 + rows * W],
                                in_=ots[h][:, c0:c0 + rows * W])
                            rr += rows
    nc.compile()
    return nc


def get_program():
    if "nc" not in _CACHE:
        _CACHE["nc"] = _build_program()
    return _CACHE["nc"]


def make_in_maps(x, weight, bias):
    x = np.asarray(x, dtype=np.float32)
    weight = np.asarray(weight, dtype=np.float32)
    bias = np.asarray(bias, dtype=np.float32)

    xpad = np.zeros((N, C_IN, HP, WP), dtype=ml_dtypes.bfloat16)
    xpad[:, :, 1:1 + H, 1:1 + W] = x.astype(ml_dtypes.bfloat16)
    w_t = np.ascontiguousarray(
        weight.transpose(1, 2, 3, 0).reshape(C_IN, KH * KW, C_OUT)
    ).astype(ml_dtypes.bfloat16)
    b2 = np.ascontiguousarray(bias.reshape(NH, 128).T)

    return [
        {
            "xp": np.ascontiguousarray(xpad[i * IMGS:(i + 1) * IMGS]),
            "w": w_t,
            "b": b2,
        }
        for i in range(N_CORES)
    ]


def kernel(x, weight, bias):
    from concourse.bass_utils import run_bass_kernel_spmd

    nc = get_program()
    in_maps = make_in_maps(x, weight, bias)
    res = run_bass_kernel_spmd(nc, in_maps, core_ids=list(range(N_CORES)))
    return np.concatenate([res.results[i]["out"] for i in range(N_CORES)],
                          axis=0)


# revision 6
# speedup vs baseline: 1.0116x; 1.0116x over previous
"""v9: bf16 conv; deadline-robust head schedule.

First matmul needs only xa00 (x rows 0..9) + w0 (k=0..2): both lead the
sync queue, which starts draining ~1us before scalar's. w1/w2 lead the
scalar queue. t=0 interleaves h0/h1 k-groups of 3 so w1 isn't needed
until ~6 matmuls in and w2 until ~12 (vs 3/6 with a straight k loop) --
v8 lost 5us to a w1 stall (+ p-state re-ramp) when output-tile DMAs
diluted the sync queue. x comes as 4 row-tiles per image (0:10, 8:18,
16:26, 24:58; overlap rows duplicated) so each conv tile reads one tile.
"""

import sys

if "/opt/trn_rl_repo" not in sys.path:
    sys.path.insert(0, "/opt/trn_rl_repo")

import ml_dtypes
import numpy as np

N, C_IN, H, W = 32, 128, 56, 56
C_OUT, KH, KW = 256, 3, 3
N_CORES = 8
IMGS = N // N_CORES
HP, WP = H + 2, W + 2
RPT = 8
NT = H // RPT          # 7
TF = RPT * W           # 448
NH = C_OUT // 128      # 2

# x row-tiles: tag -> (row0, row1); tile t reads tag via T2TAG
XTILES = {"xa0": (0, 10), "xa1": (8, 18), "xaB": (16, 26), "xb": (24, 58)}
T2TAG = {0: "xa0", 1: "xa1", 2: "xaB", 3: "xb", 4: "xb", 5: "xb", 6: "xb"}
N_WARMUP_MM = 9

_CACHE = {}


def _build_program():
    import concourse.mybir as mybir
    import concourse.tile as tile
    from concourse import bacc

    F32 = mybir.dt.float32
    BF16 = mybir.dt.bfloat16

    nc = bacc.Bacc("TRN2", target_bir_lowering=False, debug=False,
                   enable_asserts=False)

    xp = nc.dram_tensor("xp", [IMGS, C_IN, HP, WP], BF16,
                        kind="ExternalInput").ap()
    w = nc.dram_tensor("w", [C_IN, KH * KW, C_OUT], BF16,
                       kind="ExternalInput").ap()
    b = nc.dram_tensor("b", [128, NH], F32, kind="ExternalInput").ap()
    out = nc.dram_tensor("out", [IMGS, C_OUT, H, W], F32,
                         kind="ExternalOutput").ap()
    out_v = out.rearrange("n c a b -> n c (a b)")

    def xtile(pool, img, tag):
        r0, r1 = XTILES[tag]
        t = pool.tile([C_IN, r1 - r0, WP], BF16, tag=tag, name=f"x{img}_{tag}")
        return t, (lambda eng: eng.dma_start(out=t[:], in_=xp[img, :, r0:r1]))

    with tile.TileContext(nc) as tc:
        with (
            tc.tile_pool(name="consts", bufs=1) as consts,
            tc.tile_pool(name="xin", bufs=1) as xin,
            tc.tile_pool(name="outp", bufs=2) as outp,
            tc.tile_pool(name="psum", bufs=7, space="PSUM") as psum,
        ):
            scratch = consts.tile([128, TF], BF16, tag="scratch")
            nc.vector.memset(scratch[:], 0.0)

            xts = {0: {}}
            # sync queue head: exactly the first-matmul dependencies
            t0, dma0 = xtile(xin, 0, "xa0")
            dma0(nc.sync)
            xts[0]["xa0"] = t0
            w_sb = []
            for j in range(3):
                wj = consts.tile([C_IN, 3, C_OUT], BF16, tag=f"w{j}", name=f"w{j}")
                w_sb.append(wj)
            nc.sync.dma_start(out=w_sb[0][:], in_=w[:, 0:3])

            # scalar queue: w1, w2 first (needed ~1.1/2.2us after mm0),
            # then the rest in deadline order
            nc.scalar.dma_start(out=w_sb[1][:], in_=w[:, 3:6])
            nc.scalar.dma_start(out=w_sb[2][:], in_=w[:, 6:9])
            for tag in ("xa1", "xaB"):
                t_, d_ = xtile(xin, 0, tag)
                d_(nc.scalar)
                xts[0][tag] = t_
            b_sb = consts.tile([128, NH], F32, tag="b")
            nc.scalar.dma_start(out=b_sb[:], in_=b)
            t_, d_ = xtile(xin, 0, "xb")
            d_(nc.scalar)
            xts[0]["xb"] = t_
            for img in range(1, IMGS):
                xts[img] = {}
                for tag in XTILES:
                    t_, d_ = xtile(xin, img, tag)
                    d_(nc.scalar)
                    xts[img][tag] = t_

            warm_ps = psum.tile([128, TF], F32, tag="warm", bufs=1)
            for _ in range(N_WARMUP_MM):
                nc.tensor.matmul(warm_ps[:, :], lhsT=scratch[:, :128],
                                 rhs=scratch[:, :], start=True, stop=True)

            def mm(pt, src, r0, h, k, rows):
                kh, kw = divmod(k, KW)
                nc.tensor.matmul(
                    pt[:, :rows * W],
                    lhsT=w_sb[k // 3][:, k % 3, h * 128:(h + 1) * 128],
                    rhs=src[:, r0 + kh:r0 + kh + rows, kw:kw + W],
                    start=(k == 0),
                    stop=(k == KH * KW - 1),
                )

            def drain(ots, img, h, c0, pt, cols):
                nc.vector.tensor_scalar_add(
                    out=ots[h][:, c0:c0 + cols], in0=pt[:, :cols],
                    scalar1=b_sb[:, h:h + 1])
                nc.sync.dma_start(
                    out=out_v[img, h * 128:(h + 1) * 128, c0:c0 + cols],
                    in_=ots[h][:, c0:c0 + cols])

            for img in range(IMGS):
                ots = [outp.tile([128, H * W], F32, tag=f"ot{h}",
                                 name=f"ot{img}_{h}")
                       for h in range(NH)]
                for t in range(NT):
                    src = xts[img][T2TAG[t]]
                    r0 = RPT * t - XTILES[T2TAG[t]][0]
                    if t == 0:
                        # interleave h0/h1 k-groups: w1/w2 deadlines double
                        pts = [psum.tile([128, TF], F32, tag="pt",
                                          name=f"pt{img}_{t}_{h}")
                               for h in range(NH)]
                        for kc in range(3):
                            for h in range(NH):
                                for k in range(3 * kc, 3 * kc + 3):
                                    mm(pts[h], src, r0, h, k, RPT)
                        for h in range(NH):
                            drain(ots, img, h, t * TF, pts[h], TF)
                        continue
                    for h in range(NH):
                        last = (img == IMGS - 1 and t == NT - 1 and h == NH - 1)
                        row_parts = [4, 2, 2] if last else [RPT]
                        rr = 0
                        for rows in row_parts:
                            pt = psum.tile([128, rows * W], F32, tag="pt",
                                           name=f"pt{img}_{t}_{h}_{rr}")
                            for k in range(KH * KW):
                                mm(pt, src, r0 + rr, h, k, rows)
                            drain(ots, img, h, t * TF + rr * W, pt, rows * W)
                            rr += rows
    nc.compile()
    return nc


def get_program():
    if "nc" not in _CACHE:
        _CACHE["nc"] = _build_program()
    return _CACHE["nc"]


def make_in_maps(x, weight, bias):
    x = np.asarray(x, dtype=np.float32)
    weight = np.asarray(weight, dtype=np.float32)
    bias = np.asarray(bias, dtype=np.float32)

    xpad = np.zeros((N, C_IN, HP, WP), dtype=ml_dtypes.bfloat16)
    xpad[:, :, 1:1 + H, 1:1 + W] = x.astype(ml_dtypes.bfloat16)
    w_t = np.ascontiguousarray(
        weight.transpose(1, 2, 3, 0).reshape(C_IN, KH * KW, C_OUT)
    ).astype(ml_dtypes.bfloat16)
    b2 = np.ascontiguousarray(bias.reshape(NH, 128).T)

    return [
        {
            "xp": np.ascontiguousarray(xpad[i * IMGS:(i + 1) * IMGS]),
            "w": w_t,
            "b": b2,
        }
        for i in range(N_CORES)
    ]


def kernel(x, weight, bias):
    from concourse.bass_utils import run_bass_kernel_spmd

    nc = get_program()
    in_maps = make_in_maps(x, weight, bias)
    res = run_bass_kernel_spmd(nc, in_maps, core_ids=list(range(N_CORES)))
    return np.concatenate([res.results[i]["out"] for i in range(N_CORES)],
                          axis=0)


# revision 7
# speedup vs baseline: 1.0163x; 1.0047x over previous
"""v10: bf16 conv; jitter-robust head schedule.

Queue start order flips run to run (+-1us), so no tight cross-queue
deadlines: w loads as 9 per-k 64KB slices leading the sync queue (the
k=j slice is needed j*0.19us after mm0 with generous slack; t=0
interleaves h0/h1 k-groups to stretch the early ones), all x row-tiles
+ b lead the scalar queue (xa00 first; only it gates mm0). Warmup 11
matmuls ~= the 3us p-state ramp the PE needs after body entry anyway;
any PE idle gap costs ~2x (re-ramp), so warmup errs long.
"""

import sys

if "/opt/trn_rl_repo" not in sys.path:
    sys.path.insert(0, "/opt/trn_rl_repo")

import ml_dtypes
import numpy as np

N, C_IN, H, W = 32, 128, 56, 56
C_OUT, KH, KW = 256, 3, 3
N_CORES = 8
IMGS = N // N_CORES
HP, WP = H + 2, W + 2
RPT = 8
NT = H // RPT          # 7
TF = RPT * W           # 448
NH = C_OUT // 128      # 2

# x row-tiles: tag -> (row0, row1); tile t reads tag via T2TAG
XTILES = {"xa0": (0, 10), "xa1": (8, 18), "xaB": (16, 26), "xb": (24, 58)}
T2TAG = {0: "xa0", 1: "xa1", 2: "xaB", 3: "xb", 4: "xb", 5: "xb", 6: "xb"}
N_WARMUP_MM = 11

_CACHE = {}


def _build_program():
    import concourse.mybir as mybir
    import concourse.tile as tile
    from concourse import bacc

    F32 = mybir.dt.float32
    BF16 = mybir.dt.bfloat16

    nc = bacc.Bacc("TRN2", target_bir_lowering=False, debug=False,
                   enable_asserts=False)

    xp = nc.dram_tensor("xp", [IMGS, C_IN, HP, WP], BF16,
                        kind="ExternalInput").ap()
    w = nc.dram_tensor("w", [C_IN, KH * KW, C_OUT], BF16,
                       kind="ExternalInput").ap()
    b = nc.dram_tensor("b", [128, NH], F32, kind="ExternalInput").ap()
    out = nc.dram_tensor("out", [IMGS, C_OUT, H, W], F32,
                         kind="ExternalOutput").ap()
    out_v = out.rearrange("n c a b -> n c (a b)")

    def xtile(pool, img, tag):
        r0, r1 = XTILES[tag]
        t = pool.tile([C_IN, r1 - r0, WP], BF16, tag=tag, name=f"x{img}_{tag}")
        return t, (lambda eng: eng.dma_start(out=t[:], in_=xp[img, :, r0:r1]))

    with tile.TileContext(nc) as tc:
        with (
            tc.tile_pool(name="consts", bufs=1) as consts,
            tc.tile_pool(name="xin", bufs=1) as xin,
            tc.tile_pool(name="outp", bufs=2) as outp,
            tc.tile_pool(name="psum", bufs=7, space="PSUM") as psum,
        ):
            scratch = consts.tile([128, TF], BF16, tag="scratch")
            nc.vector.memset(scratch[:], 0.0)

            # sync queue: the 9 per-k w slices, then (later) out tiles
            w_sb = []
            for j in range(KH * KW):
                wj = consts.tile([C_IN, 1, C_OUT], BF16, tag=f"w{j}",
                                 name=f"w{j}")
                nc.sync.dma_start(out=wj[:], in_=w[:, j:j + 1])
                w_sb.append(wj)

            # scalar queue: x row-tiles in consumption order, b after img0
            xts = {}
            b_sb = consts.tile([128, NH], F32, tag="b")
            for img in range(IMGS):
                xts[img] = {}
                for tag in XTILES:
                    t_, d_ = xtile(xin, img, tag)
                    d_(nc.scalar)
                    xts[img][tag] = t_
                if img == 0:
                    nc.scalar.dma_start(out=b_sb[:], in_=b)

            warm_ps = psum.tile([128, TF], F32, tag="warm", bufs=1)
            for _ in range(N_WARMUP_MM):
                nc.tensor.matmul(warm_ps[:, :], lhsT=scratch[:, :128],
                                 rhs=scratch[:, :], start=True, stop=True)

            def mm(pt, src, r0, h, k, rows):
                kh, kw = divmod(k, KW)
                nc.tensor.matmul(
                    pt[:, :rows * W],
                    lhsT=w_sb[k][:, 0, h * 128:(h + 1) * 128],
                    rhs=src[:, r0 + kh:r0 + kh + rows, kw:kw + W],
                    start=(k == 0),
                    stop=(k == KH * KW - 1),
                )

            def drain(ots, img, h, c0, pt, cols):
                nc.vector.tensor_scalar_add(
                    out=ots[h][:, c0:c0 + cols], in0=pt[:, :cols],
                    scalar1=b_sb[:, h:h + 1])
                nc.sync.dma_start(
                    out=out_v[img, h * 128:(h + 1) * 128, c0:c0 + cols],
                    in_=ots[h][:, c0:c0 + cols])

            for img in range(IMGS):
                ots = [outp.tile([128, H * W], F32, tag=f"ot{h}",
                                 name=f"ot{img}_{h}")
                       for h in range(NH)]
                for t in range(NT):
                    src = xts[img][T2TAG[t]]
                    r0 = RPT * t - XTILES[T2TAG[t]][0]
                    if t == 0:
                        # interleave h0/h1 k-groups: w1/w2 deadlines double
                        pts = [psum.tile([128, TF], F32, tag="pt",
                                          name=f"pt{img}_{t}_{h}")
                               for h in range(NH)]
                        for kc in range(3):
                            for h in range(NH):
                                for k in range(3 * kc, 3 * kc + 3):
                                    mm(pts[h], src, r0, h, k, RPT)
                        for h in range(NH):
                            drain(ots, img, h, t * TF, pts[h], TF)
                        continue
                    for h in range(NH):
                        last = (img == IMGS - 1 and t == NT - 1 and h == NH - 1)
                        row_parts = [4, 2, 2] if last else [RPT]
                        rr = 0
                        for rows in row_parts:
                            pt = psum.tile([128, rows * W], F32, tag="pt",
                                           name=f"pt{img}_{t}_{h}_{rr}")
                            for k in range(KH * KW):
                                mm(pt, src, r0 + rr, h, k, rows)
                            drain(ots, img, h, t * TF + rr * W, pt, rows * W)
                            rr += rows
    nc.compile()
    return nc


def get_program():
    if "nc" not in _CACHE:
        _CACHE["nc"] = _build_program()
    return _CACHE["nc"]


def make_in_maps(x, weight, bias):
    x = np.asarray(x, dtype=np.float32)
    weight = np.asarray(weight, dtype=np.float32)
    bias = np.asarray(bias, dtype=np.float32)

    xpad = np.zeros((N, C_IN, HP, WP), dtype=ml_dtypes.bfloat16)
    xpad[:, :, 1:1 + H, 1:1 + W] = x.astype(ml_dtypes.bfloat16)
    w_t = np.ascontiguousarray(
        weight.transpose(1, 2, 3, 0).reshape(C_IN, KH * KW, C_OUT)
    ).astype(ml_dtypes.bfloat16)
    b2 = np.ascontiguousarray(bias.reshape(NH, 128).T)

    return [
        {
            "xp": np.ascontiguousarray(xpad[i * IMGS:(i + 1) * IMGS]),
            "w": w_t,
            "b": b2,
        }
        for i in range(N_CORES)
    ]


def kernel(x, weight, bias):
    from concourse.bass_utils import run_bass_kernel_spmd

    nc = get_program()
    in_maps = make_in_maps(x, weight, bias)
    res = run_bass_kernel_spmd(nc, in_maps, core_ids=list(range(N_CORES)))
    return np.concatenate([res.results[i]["out"] for i in range(N_CORES)],
                          axis=0)


# revision 8
# speedup vs baseline: 1.0390x; 1.0223x over previous
"""v10: bf16 conv; jitter-robust head schedule.

Queue start order flips run to run (+-1us), so no tight cross-queue
deadlines: w loads as 3 k-group slices leading the sync queue (9 per-k
slices lost 5.6us to serialized ~620ns DMA triggers on the sync
engine); t=0 interleaves h0/h1 k-groups so w[3:6]/w[6:9] aren't needed
until ~1.1/2.2us after mm0. All x row-tiles + b lead the scalar queue
(xa00 first; only it gates mm0). Warmup 11 matmuls ~= the 3us p-state
ramp the PE needs after body entry anyway; any PE idle gap costs ~2x
(re-ramp), so warmup errs long.
"""

import sys

if "/opt/trn_rl_repo" not in sys.path:
    sys.path.insert(0, "/opt/trn_rl_repo")

import ml_dtypes
import numpy as np

N, C_IN, H, W = 32, 128, 56, 56
C_OUT, KH, KW = 256, 3, 3
N_CORES = 8
IMGS = N // N_CORES
HP, WP = H + 2, W + 2
RPT = 8
NT = H // RPT          # 7
TF = RPT * W           # 448
NH = C_OUT // 128      # 2

# x row-tiles: tag -> (row0, row1); tile t reads tag via T2TAG
XTILES = {"xa0": (0, 10), "xa1": (8, 18), "xaB": (16, 26), "xb": (24, 58)}
T2TAG = {0: "xa0", 1: "xa1", 2: "xaB", 3: "xb", 4: "xb", 5: "xb", 6: "xb"}
N_WARMUP_MM = 11

_CACHE = {}


def _build_program():
    import concourse.mybir as mybir
    import concourse.tile as tile
    from concourse import bacc

    F32 = mybir.dt.float32
    BF16 = mybir.dt.bfloat16

    nc = bacc.Bacc("TRN2", target_bir_lowering=False, debug=False,
                   enable_asserts=False)

    xp = nc.dram_tensor("xp", [IMGS, C_IN, HP, WP], BF16,
                        kind="ExternalInput").ap()
    w = nc.dram_tensor("w", [C_IN, KH * KW, C_OUT], BF16,
                       kind="ExternalInput").ap()
    b = nc.dram_tensor("b", [128, NH], F32, kind="ExternalInput").ap()
    out = nc.dram_tensor("out", [IMGS, C_OUT, H, W], F32,
                         kind="ExternalOutput").ap()
    out_v = out.rearrange("n c a b -> n c (a b)")

    def xtile(pool, img, tag):
        r0, r1 = XTILES[tag]
        t = pool.tile([C_IN, r1 - r0, WP], BF16, tag=tag, name=f"x{img}_{tag}")
        return t, (lambda eng: eng.dma_start(out=t[:], in_=xp[img, :, r0:r1]))

    with tile.TileContext(nc) as tc:
        with (
            tc.tile_pool(name="consts", bufs=1) as consts,
            tc.tile_pool(name="xin", bufs=1) as xin,
            tc.tile_pool(name="outp", bufs=2) as outp,
            tc.tile_pool(name="psum", bufs=7, space="PSUM") as psum,
        ):
            scratch = consts.tile([128, TF], BF16, tag="scratch")
            nc.vector.memset(scratch[:], 0.0)

            # sync queue: 3 w k-group slices, then (later) out tiles
            w_sb = []
            for j in range(3):
                wj = consts.tile([C_IN, 3, C_OUT], BF16, tag=f"w{j}",
                                 name=f"w{j}")
                nc.sync.dma_start(out=wj[:], in_=w[:, 3 * j:3 * j + 3])
                w_sb.append(wj)

            # scalar queue: x row-tiles in consumption order, b after img0
            xts = {}
            b_sb = consts.tile([128, NH], F32, tag="b")
            for img in range(IMGS):
                xts[img] = {}
                for tag in XTILES:
                    t_, d_ = xtile(xin, img, tag)
                    d_(nc.scalar)
                    xts[img][tag] = t_
                if img == 0:
                    nc.scalar.dma_start(out=b_sb[:], in_=b)

            warm_ps = psum.tile([128, TF], F32, tag="warm", bufs=1)
            for _ in range(N_WARMUP_MM):
                nc.tensor.matmul(warm_ps[:, :], lhsT=scratch[:, :128],
                                 rhs=scratch[:, :], start=True, stop=True)

            def mm(pt, src, r0, h, k, rows):
                kh, kw = divmod(k, KW)
                nc.tensor.matmul(
                    pt[:, :rows * W],
                    lhsT=w_sb[k // 3][:, k % 3, h * 128:(h + 1) * 128],
                    rhs=src[:, r0 + kh:r0 + kh + rows, kw:kw + W],
                    start=(k == 0),
                    stop=(k == KH * KW - 1),
                )

            def drain(ots, img, h, c0, pt, cols):
                nc.vector.tensor_scalar_add(
                    out=ots[h][:, c0:c0 + cols], in0=pt[:, :cols],
                    scalar1=b_sb[:, h:h + 1])
                nc.sync.dma_start(
                    out=out_v[img, h * 128:(h + 1) * 128, c0:c0 + cols],
                    in_=ots[h][:, c0:c0 + cols])

            for img in range(IMGS):
                ots = [outp.tile([128, H * W], F32, tag=f"ot{h}",
                                 name=f"ot{img}_{h}")
                       for h in range(NH)]
                for t in range(NT):
                    src = xts[img][T2TAG[t]]
                    r0 = RPT * t - XTILES[T2TAG[t]][0]
                    if t == 0:
                        # interleave h0/h1 k-groups: w1/w2 deadlines double
                        pts = [psum.tile([128, TF], F32, tag="pt",
                                          name=f"pt{img}_{t}_{h}")
                               for h in range(NH)]
                        for kc in range(3):
                            for h in range(NH):
                                for k in range(3 * kc, 3 * kc + 3):
                                    mm(pts[h], src, r0, h, k, RPT)
                        for h in range(NH):
                            drain(ots, img, h, t * TF, pts[h], TF)
                        continue
                    for h in range(NH):
                        last = (img == IMGS - 1 and t == NT - 1 and h == NH - 1)
                        row_parts = [4, 4] if last else [RPT]
                        rr = 0
                        for rows in row_parts:
                            pt = psum.tile([128, rows * W], F32, tag="pt",
                                           name=f"pt{img}_{t}_{h}_{rr}")
                            for k in range(KH * KW):
                                mm(pt, src, r0 + rr, h, k, rows)
                            drain(ots, img, h, t * TF + rr * W, pt, rows * W)
                            rr += rows
    nc.compile()
    return nc


def get_program():
    if "nc" not in _CACHE:
        _CACHE["nc"] = _build_program()
    return _CACHE["nc"]


def make_in_maps(x, weight, bias):
    x = np.asarray(x, dtype=np.float32)
    weight = np.asarray(weight, dtype=np.float32)
    bias = np.asarray(bias, dtype=np.float32)

    xpad = np.zeros((N, C_IN, HP, WP), dtype=ml_dtypes.bfloat16)
    xpad[:, :, 1:1 + H, 1:1 + W] = x.astype(ml_dtypes.bfloat16)
    w_t = np.ascontiguousarray(
        weight.transpose(1, 2, 3, 0).reshape(C_IN, KH * KW, C_OUT)
    ).astype(ml_dtypes.bfloat16)
    b2 = np.ascontiguousarray(bias.reshape(NH, 128).T)

    return [
        {
            "xp": np.ascontiguousarray(xpad[i * IMGS:(i + 1) * IMGS]),
            "w": w_t,
            "b": b2,
        }
        for i in range(N_CORES)
    ]


def kernel(x, weight, bias):
    from concourse.bass_utils import run_bass_kernel_spmd

    nc = get_program()
    in_maps = make_in_maps(x, weight, bias)
    res = run_bass_kernel_spmd(nc, in_maps, core_ids=list(range(N_CORES)))
    return np.concatenate([res.results[i]["out"] for i in range(N_CORES)],
                          axis=0)
